# revision 8
# baseline (speedup 1.0000x reference)
"""Sliding-window softcapped GQA attention, tensor-parallel across 8 NeuronCores.

Sharding (per spec hint): core c owns KV head c and Q heads 4c..4c+3.
Each core computes x->q/k/v proj, QK-RMSNorm, RoPE, windowed softcapped
attention, and its partial o_proj; host sums the 8 partial outputs.

Layouts: everything lives transposed on device ([feature, token]) so every
matmul contracts over the partition dim with zero on-device transposes of x/w
(host pre-transposes). Matmuls run in float32r (full PE rate, ~1e-4 rel err).
"""
import numpy as np

B, S, HID = 2, 2048, 4096
NQ, NK, HD = 32, 8, 128
WINDOW = 1024
SOFTCAP = 50.0
EPS = 1e-6
NCORES = 8
QD = NQ // NCORES * HD      # 512 q-dims per core
TOK = B * S                 # 4096 tokens
NBLK = 4                    # q-blocks of 512 per batch
BLK = 512
KTILES = HID // 128         # 32 k tiles over hidden
NH = NQ // NCORES           # 4 q heads per core

_CACHE = {}


def _window_jts(qb):
    lo = max(0, qb * BLK - (WINDOW - 1)) // 128
    hi = (qb * BLK + BLK - 1) // 128
    return lo, hi


def _tile_mask_kind(qb, jt):
    """None = fully allowed, 'causal' or 'window' = needs affine mask."""
    if jt * 128 + 127 > qb * BLK:
        return "causal"
    if jt * 128 < qb * BLK - BLK:
        return "window"
    return None


def _build():
    import concourse.bass as bass
    import concourse.mybir as mybir
    import concourse.tile as tile
    from concourse import bacc
    from concourse.masks import make_identity

    f32, f32r, bf16 = mybir.dt.float32, mybir.dt.float32r, mybir.dt.bfloat16
    AF = mybir.ActivationFunctionType
    ALU = mybir.AluOpType

    nc = bacc.Bacc("TRN2", target_bir_lowering=False, debug=False,
                   num_devices=NCORES)

    # ---- DRAM I/O (per-core shapes; same program on all cores) ----
    # slabs: per (block, ktile): [128 hid, 512 x | 128 wk | 128 wv]
    slabs = nc.dram_tensor("slabs", (B * NBLK * KTILES * 128, 768), f32r,
                           kind="ExternalInput").ap()
    wqT = nc.dram_tensor("wqT", (HID, QD), f32r, kind="ExternalInput").ap()
    woT = nc.dram_tensor("woT", (QD, HID), bf16, kind="ExternalInput").ap()
    cosT = nc.dram_tensor("cosT", (128, S), f32, kind="ExternalInput").ap()
    sinT = nc.dram_tensor("sinT", (128, S), f32, kind="ExternalInput").ap()
    qnw = nc.dram_tensor("qnw", (128, 1), f32, kind="ExternalInput").ap()
    knw = nc.dram_tensor("knw", (128, 1), f32, kind="ExternalInput").ap()
    outT = nc.dram_tensor("outT", (HID, TOK), f32, kind="ExternalOutput").ap()
    oT_stash = nc.dram_tensor("oT_stash", (QD, TOK), bf16).ap()

    with tile.TileContext(nc) as tc:
        with tc.tile_pool(name="wts", bufs=1) as wts, \
             tc.tile_pool(name="stream", bufs=3) as stream, \
             tc.tile_pool(name="persist", bufs=1) as persist, \
             tc.tile_pool(name="work", bufs=2) as work, \
             tc.tile_pool(name="pwork", bufs=4) as pwork, \
             tc.tile_pool(name="stage", bufs=2) as stage, \
             tc.tile_pool(name="ps", bufs=8, space="PSUM") as ps:

            # ---- resident constants / weights ----
            wq_s = wts.tile([128, KTILES * QD], f32r)       # 64KB/p
            for k in range(KTILES):
                nc.sync.dma_start(wq_s[:, k * QD:(k + 1) * QD],
                                  wqT[k * 128:(k + 1) * 128, :])
            wo_s = wts.tile([128, 4 * HID], bf16)           # 32KB/p
            for kk in range(4):
                nc.sync.dma_start(wo_s[:, kk * HID:(kk + 1) * HID],
                                  woT[kk * 128:(kk + 1) * 128, :])
            cos2 = wts.tile([128, S], f32)
            sin2 = wts.tile([128, S], f32)
            nc.sync.dma_start(cos2[:], cosT[:])
            nc.sync.dma_start(sin2[:], sinT[:])
            qnw_s = wts.tile([128, 1], f32)
            knw_s = wts.tile([128, 1], f32)
            nc.sync.dma_start(qnw_s[:], qnw[:])
            nc.sync.dma_start(knw_s[:], knw[:])

            ones_f = wts.tile([128, 1], f32)
            nc.gpsimd.memset(ones_f[:], 1.0)
            ones_r = wts.tile([128, 1], f32r)               # colsum lhsT
            nc.vector.tensor_copy(ones_r[:], ones_f[:])
            onesrow_f = wts.tile([1, 128], f32)
            nc.gpsimd.memset(onesrow_f[:], 1.0)
            onesrow_r = wts.tile([1, 128], f32r)            # bcast lhsT
            nc.vector.tensor_copy(onesrow_r[:], onesrow_f[:])
            neg50 = wts.tile([128, 1], f32)
            nc.gpsimd.memset(neg50[:], -50.0)
            eps_q = wts.tile([1, 1], f32)
            nc.gpsimd.memset(eps_q[:], EPS * HD * (SOFTCAP ** 2))
            eps_k = wts.tile([1, 1], f32)
            nc.gpsimd.memset(eps_k[:], EPS)
            ident_f = wts.tile([128, 128], f32)
            make_identity(nc, ident_f[:])
            ident_r = wts.tile([128, 128], f32r)
            nc.vector.tensor_copy(ident_r[:], ident_f[:])

            # half-swap permutation: swap[i, j] = 1 iff j == (i+64) % 128
            swap_f = wts.tile([128, 128], f32)
            nc.gpsimd.memset(swap_f[:], 0.0)
            nc.gpsimd.affine_select(out=swap_f[:], in_=swap_f[:],
                                    compare_op=ALU.not_equal, fill=1.0,
                                    base=64, pattern=[[-1, 128]],
                                    channel_multiplier=1)
            nc.gpsimd.affine_select(out=swap_f[:], in_=swap_f[:],
                                    compare_op=ALU.not_equal, fill=1.0,
                                    base=-64, pattern=[[-1, 128]],
                                    channel_multiplier=1)
            swap_r = wts.tile([128, 128], f32r)
            nc.vector.tensor_copy(swap_r[:], swap_f[:])

            def norm_rope_store(src_ps, dst, dst_col, tokpos, which):
                """Per-head RMSNorm + RoPE: src_ps [128, BLK] psum -> dst f32r."""
                sq = work.tile([128, BLK], f32r, tag="sq")
                nc.scalar.activation(sq[:], src_ps[:], AF.Square)
                cs = ps.tile([1, BLK], f32, tag="ps")
                nc.tensor.matmul(cs[:], ones_r[:], sq[:], start=True, stop=True)
                std = work.tile([1, BLK], f32, tag="std")
                if which == "q":
                    # 1/sqrt(2500*sum + 2500*128*eps) = rsqrt(var+eps)/(sqrt(hd)*50)
                    nc.scalar.activation(std[:], cs[:], AF.Sqrt,
                                         scale=SOFTCAP ** 2, bias=eps_q[:1])
                else:
                    nc.scalar.activation(std[:], cs[:], AF.Sqrt,
                                         scale=1.0 / HD, bias=eps_k[:1])
                rq = work.tile([1, BLK], f32r, tag="rq")
                with nc.allow_low_precision(reason="f32r recip for bcast matmul"):
                    nc.vector.reciprocal(rq[:], std[:])
                bc = ps.tile([128, BLK], f32, tag="ps")
                nc.tensor.matmul(bc[:], onesrow_r[:], rq[:], start=True, stop=True)
                bc_s = work.tile([128, BLK], f32, tag="bcs", bufs=1)
                nc.vector.tensor_copy(bc_s[:], bc[:])
                nrm = work.tile([128, BLK], f32r, tag="nrm")
                nc.vector.scalar_tensor_tensor(
                    nrm[:], src_ps[:], qnw_s[:] if which == "q" else knw_s[:],
                    bc_s[:], ALU.mult, ALU.mult)
                # RoPE: dst = nrm * [cos;cos] + swap(nrm) * [-sin;sin]
                rot = ps.tile([128, BLK], f32, tag="ps")
                nc.tensor.matmul(rot[:], swap_r[:], nrm[:], start=True, stop=True)
                m1 = work.tile([128, BLK], f32, tag="r1", bufs=1)
                nc.vector.tensor_mul(m1[:], nrm[:], cos2[:, tokpos:tokpos + BLK])
                m2 = work.tile([128, BLK], f32, tag="r2", bufs=1)
                nc.vector.tensor_mul(m2[:], rot[:], sin2[:, tokpos:tokpos + BLK])
                nc.vector.tensor_add(dst[:, dst_col:dst_col + BLK], m1[:], m2[:])

            for b in range(B):
                khat = persist.tile([128, S], f32r, tag="khat")
                vnat = persist.tile([128, S], f32r, tag="vnat")
                for qb in range(NBLK):
                    tok0 = b * S + qb * BLK
                    pos0 = qb * BLK
                    # ---- projections over hidden k-tiles ----
                    qps = [ps.tile([128, BLK], f32, tag="ps", name=f"qps{m}") for m in range(NH)]
                    kps = ps.tile([128, BLK], f32, tag="ps")
                    vps = ps.tile([128, BLK], f32, tag="ps")
                    for k in range(KTILES):
                        row0 = ((b * NBLK + qb) * KTILES + k) * 128
                        sl = stream.tile([128, 768], f32r, tag="slab")
                        nc.sync.dma_start(sl[:], slabs[row0:row0 + 128, :])
                        xt = sl[:, 0:512]
                        for m in range(NH):
                            nc.tensor.matmul(
                                qps[m][:], wq_s[:, k * QD + m * 128: k * QD + (m + 1) * 128],
                                xt, start=(k == 0), stop=(k == KTILES - 1))
                        nc.tensor.matmul(kps[:], sl[:, 512:640], xt,
                                         start=(k == 0), stop=(k == KTILES - 1))
                        nc.tensor.matmul(vps[:], sl[:, 640:768], xt,
                                         start=(k == 0), stop=(k == KTILES - 1))
                    # ---- norm + rope ----
                    qhat = work.tile([128, NH * BLK], f32r, tag="qhat", bufs=1)
                    for m in range(NH):
                        norm_rope_store(qps[m], qhat, m * BLK, pos0, "q")
                    norm_rope_store(kps, khat, qb * BLK, pos0, "k")
                    # ---- v: psum [128 vd, BLK tok] -> natural [tok, vd] ----
                    vT_s = work.tile([128, BLK], f32r, tag="vTs")
                    nc.vector.tensor_copy(vT_s[:], vps[:])
                    for tt in range(4):
                        vtr = ps.tile([128, 128], f32r, tag="ps")
                        nc.tensor.transpose(vtr[:], vT_s[:, tt * 128:(tt + 1) * 128],
                                            ident_r[:])
                        nc.vector.tensor_copy(
                            vnat[:, qb * BLK + tt * 128: qb * BLK + (tt + 1) * 128],
                            vtr[:])
                    # ---- attention per head ----
                    lo, hi = _window_jts(qb)
                    for h in range(NH):
                        qh = qhat[:, h * BLK:(h + 1) * BLK]
                        sums = ps.tile([1, BLK], f32, tag="ps")
                        ops = ps.tile([128, BLK], f32, tag="ps")
                        first = True
                        for jt in range(lo, hi + 1):
                            sps = ps.tile([128, BLK], f32, tag="ps")
                            nc.tensor.matmul(sps[:],
                                             khat[:, jt * 128:(jt + 1) * 128],
                                             qh, start=True, stop=True)
                            th = work.tile([128, BLK], f32, tag="tanh")
                            nc.scalar.activation(th[:], sps[:], AF.Tanh)
                            kind = _tile_mask_kind(qb, jt)
                            if kind == "causal":
                                nc.gpsimd.affine_select(
                                    out=th[:], in_=th[:],
                                    compare_op=ALU.is_ge, fill=-1e9,
                                    base=qb * BLK - jt * 128,
                                    pattern=[[1, BLK]], channel_multiplier=-1)
                            elif kind == "window":
                                nc.gpsimd.affine_select(
                                    out=th[:], in_=th[:],
                                    compare_op=ALU.is_ge, fill=-1e9,
                                    base=jt * 128 - qb * BLK + (WINDOW - 1),
                                    pattern=[[-1, BLK]], channel_multiplier=1)
                            pt = pwork.tile([128, BLK], f32r, tag="pt")
                            nc.scalar.activation(pt[:], th[:], AF.Exp,
                                                 scale=SOFTCAP, bias=neg50[:])
                            last = (jt == hi)
                            nc.tensor.matmul(sums[:], ones_r[:], pt[:],
                                             start=first, stop=last)
                            nc.tensor.matmul(ops[:],
                                             vnat[:, jt * 128:(jt + 1) * 128],
                                             pt[:], start=first, stop=last)
                            first = False
                        rs = work.tile([1, BLK], f32r, tag="rs")
                        with nc.allow_low_precision(reason="f32r recip for bcast matmul"):
                            nc.vector.reciprocal(rs[:], sums[:])
                        bco = ps.tile([128, BLK], f32, tag="ps")
                        nc.tensor.matmul(bco[:], onesrow_r[:], rs[:],
                                         start=True, stop=True)
                        bco_s = work.tile([128, BLK], f32, tag="bcs", bufs=1)
                        nc.vector.tensor_copy(bco_s[:], bco[:])
                        oth = stage.tile([128, BLK], bf16, tag="oth")
                        nc.vector.tensor_mul(oth[:], ops[:], bco_s[:])
                        nc.sync.dma_start(
                            oT_stash[h * 128:(h + 1) * 128, tok0:tok0 + BLK],
                            oth[:])

            # ---- phase 2: partial o_proj: outT = woT.T @ oT ----
            for n in range(8):
                otn = [stream.tile([128, BLK], bf16, tag="otn", name=f"otn{kk}", bufs=8) for kk in range(4)]
                for kk in range(4):
                    nc.sync.dma_start(otn[kk][:],
                                      oT_stash[kk * 128:(kk + 1) * 128,
                                               n * BLK:(n + 1) * BLK])
                for m in range(KTILES):
                    op2 = ps.tile([128, BLK], f32, tag="ps")
                    for kk in range(4):
                        nc.tensor.matmul(
                            op2[:], wo_s[:, kk * HID + m * 128: kk * HID + (m + 1) * 128],
                            otn[kk][:], start=(kk == 0), stop=(kk == 3))
                    og = stage.tile([128, BLK], f32, tag="og")
                    nc.vector.tensor_copy(og[:], op2[:])
                    nc.sync.dma_start(
                        outT[m * 128:(m + 1) * 128, n * BLK:(n + 1) * BLK],
                        og[:])

    nc.compile()
    return nc


def _host_inputs(x, wq, wk, wv, wo, q_norm_w, k_norm_w):
    """Build per-core input maps (host-side sharding + layout transforms)."""
    xT = np.ascontiguousarray(x.reshape(TOK, HID).T)  # [HID, TOK]

    inv_freq = 1.0 / (10000.0 ** (np.arange(0, HD, 2, dtype=np.float32) / HD))
    freqs = np.arange(S, dtype=np.float32)[:, None] * inv_freq  # [S, 64]
    c = np.cos(freqs).T.astype(np.float32)   # [64, S]
    sn = np.sin(freqs).T.astype(np.float32)
    cosT = np.ascontiguousarray(np.concatenate([c, c], axis=0))       # [cos;cos]
    sinT = np.ascontiguousarray(np.concatenate([-sn, sn], axis=0))    # [-sin;sin]

    import ml_dtypes
    in_maps = []
    for c in range(NCORES):
        wq_c = wq[c * QD:(c + 1) * QD, :]          # [512, HID]
        wk_c = wk[c * HD:(c + 1) * HD, :]          # [128, HID]
        wv_c = wv[c * HD:(c + 1) * HD, :]          # [128, HID]
        wo_c = wo[:, c * QD:(c + 1) * QD]          # [HID, 512]

        # slabs: per (block, ktile) rows [128 hid] x cols [x 512 | wk 128 | wv 128]
        slab = np.empty((B * NBLK * KTILES * 128, 768), np.float32)
        wkT_c = wk_c.T  # [HID, 128]
        wvT_c = wv_c.T
        for blk in range(B * NBLK):
            t0 = blk * BLK
            for k in range(KTILES):
                r0 = (blk * KTILES + k) * 128
                slab[r0:r0 + 128, 0:512] = xT[k * 128:(k + 1) * 128, t0:t0 + BLK]
                slab[r0:r0 + 128, 512:640] = wkT_c[k * 128:(k + 1) * 128, :]
                slab[r0:r0 + 128, 640:768] = wvT_c[k * 128:(k + 1) * 128, :]

        in_maps.append({
            "slabs": slab,
            "wqT": np.ascontiguousarray(wq_c.T),
            "woT": np.ascontiguousarray(wo_c.T).astype(ml_dtypes.bfloat16),
            "cosT": cosT, "sinT": sinT,
            "qnw": q_norm_w.reshape(128, 1).astype(np.float32),
            "knw": k_norm_w.reshape(128, 1).astype(np.float32),
        })
    return in_maps


def kernel(x, wq, wk, wv, wo, q_norm_w, k_norm_w, _trace=False):
    from concourse import bass_utils

    x = np.asarray(x, np.float32)
    wq, wk, wv, wo = (np.asarray(a, np.float32) for a in (wq, wk, wv, wo))
    q_norm_w = np.asarray(q_norm_w, np.float32)
    k_norm_w = np.asarray(k_norm_w, np.float32)

    if "nc" not in _CACHE:
        _CACHE["nc"] = _build()
    nc = _CACHE["nc"]

    in_maps = _host_inputs(x, wq, wk, wv, wo, q_norm_w, k_norm_w)
    res = bass_utils.run_bass_kernel_spmd(
        nc, in_maps, core_ids=list(range(NCORES)), trace=_trace)
    _CACHE["last_result"] = res

    acc = np.zeros((HID, TOK), np.float64)
    for c in range(NCORES):
        acc += res.results[c]["outT"].astype(np.float64)
    out = acc.astype(np.float32).T.reshape(B, S, HID)
    return out


# revision 12
# speedup vs baseline: 1.0456x; 1.0456x over previous
"""Sliding-window softcapped GQA attention, tensor-parallel across 8 NeuronCores.

Sharding (per spec hint): core c owns KV head c and Q heads 4c..4c+3.
Each core computes x->q/k/v proj, QK-RMSNorm, RoPE, windowed softcapped
attention, and its partial o_proj; host sums the 8 partial outputs.

Layouts: everything lives transposed on device ([feature, token]) so every
matmul contracts over the partition dim with zero on-device transposes of x/w
(host pre-transposes). Matmuls run in float32r (full PE rate, ~1e-4 rel err).
"""
import numpy as np

B, S, HID = 2, 2048, 4096
NQ, NK, HD = 32, 8, 128
WINDOW = 1024
SOFTCAP = 50.0
EPS = 1e-6
NCORES = 8
QD = NQ // NCORES * HD      # 512 q-dims per core
TOK = B * S                 # 4096 tokens
NBLK = 4                    # q-blocks of 512 per batch
BLK = 512
KTILES = HID // 128         # 32 k tiles over hidden
NH = NQ // NCORES           # 4 q heads per core

_CACHE = {}


def _window_jts(qb):
    lo = max(0, qb * BLK - (WINDOW - 1)) // 128
    hi = (qb * BLK + BLK - 1) // 128
    return lo, hi


def _tile_mask_kind(qb, jt):
    """None = fully allowed, 'causal' or 'window' = needs affine mask."""
    if jt * 128 + 127 > qb * BLK:
        return "causal"
    if jt * 128 < qb * BLK - BLK:
        return "window"
    return None


def _build():
    import concourse.bass as bass
    import concourse.mybir as mybir
    import concourse.tile as tile
    from concourse import bacc
    from concourse.masks import make_identity

    f32, f32r, bf16 = mybir.dt.float32, mybir.dt.float32r, mybir.dt.bfloat16
    AF = mybir.ActivationFunctionType
    ALU = mybir.AluOpType

    nc = bacc.Bacc("TRN2", target_bir_lowering=False, debug=False,
                   num_devices=NCORES)

    # ---- DRAM I/O (per-core shapes; same program on all cores) ----
    # slabs: per (block, ktile): [128 hid, 512 x | 128 wk | 128 wv]
    slabs = nc.dram_tensor("slabs", (B * NBLK * KTILES * 128, 768), f32r,
                           kind="ExternalInput").ap()
    wqT = nc.dram_tensor("wqT", (HID, QD), f32r, kind="ExternalInput").ap()
    woT = nc.dram_tensor("woT", (QD, HID), bf16, kind="ExternalInput").ap()
    cosT = nc.dram_tensor("cosT", (128, S), f32, kind="ExternalInput").ap()
    sinT = nc.dram_tensor("sinT", (128, S), f32, kind="ExternalInput").ap()
    qnw = nc.dram_tensor("qnw", (128, 1), f32, kind="ExternalInput").ap()
    knw = nc.dram_tensor("knw", (128, 1), f32, kind="ExternalInput").ap()
    outT = nc.dram_tensor("outT", (HID, TOK), f32, kind="ExternalOutput").ap()
    oT_stash = nc.dram_tensor("oT_stash", (QD, TOK), bf16).ap()

    with tile.TileContext(nc) as tc:
        with tc.tile_pool(name="wts", bufs=1) as wts, \
             tc.tile_pool(name="stream", bufs=3) as stream, \
             tc.tile_pool(name="persist", bufs=1) as persist, \
             tc.tile_pool(name="work", bufs=2) as work, \
             tc.tile_pool(name="pwork", bufs=4) as pwork, \
             tc.tile_pool(name="stage", bufs=2) as stage, \
             tc.tile_pool(name="ps", bufs=2, space="PSUM") as ps:

            # ---- resident constants / weights ----
            wq_s = wts.tile([128, KTILES * QD], f32r)       # 64KB/p
            for k in range(KTILES):
                nc.sync.dma_start(wq_s[:, k * QD:(k + 1) * QD],
                                  wqT[k * 128:(k + 1) * 128, :])
            wo_s = wts.tile([128, 4 * HID], bf16)           # 32KB/p
            for kk in range(4):
                nc.sync.dma_start(wo_s[:, kk * HID:(kk + 1) * HID],
                                  woT[kk * 128:(kk + 1) * 128, :])
            cos2 = wts.tile([128, S], f32)
            sin2 = wts.tile([128, S], f32)
            nc.sync.dma_start(cos2[:], cosT[:])
            nc.sync.dma_start(sin2[:], sinT[:])
            qnw_s = wts.tile([128, 1], f32)
            knw_s = wts.tile([128, 1], f32)
            nc.sync.dma_start(qnw_s[:], qnw[:])
            nc.sync.dma_start(knw_s[:], knw[:])

            ones_f = wts.tile([128, 1], f32)
            nc.gpsimd.memset(ones_f[:], 1.0)
            ones_r = wts.tile([128, 1], f32r)               # colsum lhsT
            nc.vector.tensor_copy(ones_r[:], ones_f[:])
            ones_b = wts.tile([128, 1], bf16)               # colsum lhsT (bf16 sq)
            nc.gpsimd.memset(ones_b[:], 1.0)
            onesrow_f = wts.tile([1, 128], f32)
            nc.gpsimd.memset(onesrow_f[:], 1.0)
            onesrow_r = wts.tile([1, 128], f32r)            # bcast lhsT
            nc.vector.tensor_copy(onesrow_r[:], onesrow_f[:])
            neg50 = wts.tile([128, 1], f32)
            nc.gpsimd.memset(neg50[:], -50.0)
            eps_q = wts.tile([1, 1], f32)
            nc.gpsimd.memset(eps_q[:], EPS * HD * (SOFTCAP ** 2))
            eps_k = wts.tile([1, 1], f32)
            nc.gpsimd.memset(eps_k[:], EPS)
            ident_f = wts.tile([128, 128], f32)
            make_identity(nc, ident_f[:])
            ident_r = wts.tile([128, 128], f32r)
            nc.vector.tensor_copy(ident_r[:], ident_f[:])

            # half-swap permutation: swap[i, j] = 1 iff j == (i+64) % 128
            swap_f = wts.tile([128, 128], f32)
            nc.gpsimd.memset(swap_f[:], 0.0)
            nc.gpsimd.affine_select(out=swap_f[:], in_=swap_f[:],
                                    compare_op=ALU.not_equal, fill=1.0,
                                    base=64, pattern=[[-1, 128]],
                                    channel_multiplier=1)
            nc.gpsimd.affine_select(out=swap_f[:], in_=swap_f[:],
                                    compare_op=ALU.not_equal, fill=1.0,
                                    base=-64, pattern=[[-1, 128]],
                                    channel_multiplier=1)
            swap_r = wts.tile([128, 128], f32r)
            nc.vector.tensor_copy(swap_r[:], swap_f[:])

            def norm_rope_store(src_ps, dst, dst_col, tokpos, which):
                """Per-head RMSNorm + RoPE: src_ps [128, BLK] psum -> dst f32r."""
                sq = work.tile([128, BLK], bf16, tag="sq", bufs=4)
                nc.scalar.activation(sq[:], src_ps, AF.Square)
                cs = ps.tile([1, BLK], f32, tag="small")
                nc.tensor.matmul(cs[:], ones_b[:], sq[:], start=True, stop=True)
                std = work.tile([1, BLK], f32, tag="std")
                if which == "q":
                    # 1/sqrt(2500*sum + 2500*128*eps) = rsqrt(var+eps)/(sqrt(hd)*50)
                    nc.scalar.activation(std[:], cs[:], AF.Sqrt,
                                         scale=SOFTCAP ** 2, bias=eps_q[:1])
                else:
                    nc.scalar.activation(std[:], cs[:], AF.Sqrt,
                                         scale=1.0 / HD, bias=eps_k[:1])
                rq = work.tile([1, BLK], f32r, tag="rq")
                with nc.allow_low_precision(reason="f32r recip for bcast matmul"):
                    nc.vector.reciprocal(rq[:], std[:])
                bc = ps.tile([128, BLK], f32, tag="small")
                nc.tensor.matmul(bc[:], onesrow_r[:], rq[:], start=True, stop=True)
                bc_s = work.tile([128, BLK], f32, tag="bcs", bufs=2)
                nc.vector.tensor_copy(bc_s[:], bc[:])
                nrm = work.tile([128, BLK], f32r, tag="nrm")
                nc.vector.scalar_tensor_tensor(
                    nrm[:], src_ps, qnw_s[:] if which == "q" else knw_s[:],
                    bc_s[:], ALU.mult, ALU.mult)
                # RoPE: dst = nrm * [cos;cos] + swap(nrm) * [-sin;sin]
                rot = ps.tile([128, BLK], f32, tag="small")
                nc.tensor.matmul(rot[:], swap_r[:], nrm[:], start=True, stop=True)
                m1 = work.tile([128, BLK], f32, tag="r1", bufs=1)
                nc.vector.tensor_mul(m1[:], nrm[:], cos2[:, tokpos:tokpos + BLK])
                m2 = work.tile([128, BLK], f32, tag="r2", bufs=1)
                nc.vector.tensor_mul(m2[:], rot[:], sin2[:, tokpos:tokpos + BLK])
                nc.vector.tensor_add(dst[:, dst_col:dst_col + BLK], m1[:], m2[:])

            for b in range(B):
                khat = persist.tile([128, S], f32r, tag="khat")
                vnat = persist.tile([128, S], f32r, tag="vnat")
                for qb in range(NBLK):
                    tok0 = b * S + qb * BLK
                    pos0 = qb * BLK
                    # ---- projections over hidden k-tiles (paired psum) ----
                    qp01 = ps.tile([128, 2 * BLK], f32, tag="big", bufs=3)
                    qp23 = ps.tile([128, 2 * BLK], f32, tag="big", bufs=3)
                    kvp = ps.tile([128, 2 * BLK], f32, tag="big", bufs=3)
                    qsl = [qp01[:, 0:BLK], qp01[:, BLK:2 * BLK],
                           qp23[:, 0:BLK], qp23[:, BLK:2 * BLK]]
                    ksl, vsl = kvp[:, 0:BLK], kvp[:, BLK:2 * BLK]
                    for k in range(KTILES):
                        row0 = ((b * NBLK + qb) * KTILES + k) * 128
                        sl = stream.tile([128, 768], f32r, tag="slab")
                        nc.sync.dma_start(sl[:], slabs[row0:row0 + 128, :])
                        xt = sl[:, 0:512]
                        for m in range(NH):
                            nc.tensor.matmul(
                                qsl[m], wq_s[:, k * QD + m * 128: k * QD + (m + 1) * 128],
                                xt, start=(k == 0), stop=(k == KTILES - 1))
                        nc.tensor.matmul(ksl, sl[:, 512:640], xt,
                                         start=(k == 0), stop=(k == KTILES - 1))
                        nc.tensor.matmul(vsl, sl[:, 640:768], xt,
                                         start=(k == 0), stop=(k == KTILES - 1))
                    # ---- norm + rope ----
                    qhat = work.tile([128, NH * BLK], f32r, tag="qhat", bufs=1)
                    for m in range(NH):
                        norm_rope_store(qsl[m], qhat, m * BLK, pos0, "q")
                    norm_rope_store(ksl, khat, qb * BLK, pos0, "k")
                    # ---- v: psum [128 vd, BLK tok] -> natural [tok, vd] ----
                    vT_s = work.tile([128, BLK], f32r, tag="vTs", bufs=1)
                    nc.vector.tensor_copy(vT_s[:], vsl)
                    for tt in range(4):
                        vtr = ps.tile([128, 128], f32r, tag="small")
                        nc.tensor.transpose(vtr[:], vT_s[:, tt * 128:(tt + 1) * 128],
                                            ident_r[:])
                        nc.vector.tensor_copy(
                            vnat[:, qb * BLK + tt * 128: qb * BLK + (tt + 1) * 128],
                            vtr[:])
                    # ---- attention per head (paired jt tiles) ----
                    lo, hi = _window_jts(qb)
                    for h in range(NH):
                        qh = qhat[:, h * BLK:(h + 1) * BLK]
                        sums = ps.tile([1, BLK], f32, tag="small")
                        ops = ps.tile([128, BLK], f32, tag="small")
                        for jp in range(lo, hi + 1, 2):
                            sp2 = ps.tile([128, 2 * BLK], f32, tag="big", bufs=3)
                            for half, jt in enumerate((jp, jp + 1)):
                                nc.tensor.matmul(
                                    sp2[:, half * BLK:(half + 1) * BLK],
                                    khat[:, jt * 128:(jt + 1) * 128],
                                    qh, start=True, stop=True)
                            th = work.tile([128, 2 * BLK], f32, tag="tanh", bufs=2)
                            nc.scalar.activation(th[:], sp2[:], AF.Tanh)
                            for half, jt in enumerate((jp, jp + 1)):
                                kind = _tile_mask_kind(qb, jt)
                                hsl = th[:, half * BLK:(half + 1) * BLK]
                                if kind == "causal":
                                    nc.gpsimd.affine_select(
                                        out=hsl, in_=hsl,
                                        compare_op=ALU.is_ge, fill=-1e9,
                                        base=qb * BLK - jt * 128,
                                        pattern=[[1, BLK]], channel_multiplier=-1)
                                elif kind == "window":
                                    nc.gpsimd.affine_select(
                                        out=hsl, in_=hsl,
                                        compare_op=ALU.is_ge, fill=-1e9,
                                        base=jt * 128 - qb * BLK + (WINDOW - 1),
                                        pattern=[[-1, BLK]], channel_multiplier=1)
                            pt = pwork.tile([128, 2 * BLK], f32r, tag="pt", bufs=2)
                            nc.scalar.activation(pt[:], th[:], AF.Exp,
                                                 scale=SOFTCAP, bias=neg50[:])
                            for half, jt in enumerate((jp, jp + 1)):
                                psl = pt[:, half * BLK:(half + 1) * BLK]
                                nc.tensor.matmul(sums[:], ones_r[:], psl,
                                                 start=(jt == lo), stop=(jt == hi))
                                nc.tensor.matmul(ops[:],
                                                 vnat[:, jt * 128:(jt + 1) * 128],
                                                 psl, start=(jt == lo), stop=(jt == hi))
                        rs = work.tile([1, BLK], f32r, tag="rs", bufs=1)
                        with nc.allow_low_precision(reason="f32r recip for bcast matmul"):
                            nc.vector.reciprocal(rs[:], sums[:])
                        bco = ps.tile([128, BLK], f32, tag="small")
                        nc.tensor.matmul(bco[:], onesrow_r[:], rs[:],
                                         start=True, stop=True)
                        bco_s = work.tile([128, BLK], f32, tag="bcs", bufs=2)
                        nc.vector.tensor_copy(bco_s[:], bco[:])
                        oth = stage.tile([128, BLK], bf16, tag="oth")
                        nc.vector.tensor_mul(oth[:], ops[:], bco_s[:])
                        nc.sync.dma_start(
                            oT_stash[h * 128:(h + 1) * 128, tok0:tok0 + BLK],
                            oth[:])

            # ---- phase 2: partial o_proj: outT = woT.T @ oT ----
            for n in range(0, 8, 2):
                otn = [stream.tile([128, 2 * BLK], bf16, tag="otn",
                                   name=f"otn{kk}", bufs=4) for kk in range(4)]
                for kk in range(4):
                    nc.sync.dma_start(otn[kk][:],
                                      oT_stash[kk * 128:(kk + 1) * 128,
                                               n * BLK:(n + 2) * BLK])
                for m in range(KTILES):
                    op2 = ps.tile([128, 2 * BLK], f32, tag="big", bufs=3)
                    for half in range(2):
                        for kk in range(4):
                            nc.tensor.matmul(
                                op2[:, half * BLK:(half + 1) * BLK],
                                wo_s[:, kk * HID + m * 128: kk * HID + (m + 1) * 128],
                                otn[kk][:, half * BLK:(half + 1) * BLK],
                                start=(kk == 0), stop=(kk == 3))
                    og = stage.tile([128, 2 * BLK], f32, tag="og", bufs=1)
                    nc.vector.tensor_copy(og[:], op2[:])
                    nc.sync.dma_start(
                        outT[m * 128:(m + 1) * 128, n * BLK:(n + 2) * BLK],
                        og[:])

    nc.compile()
    return nc


def _host_inputs(x, wq, wk, wv, wo, q_norm_w, k_norm_w):
    """Build per-core input maps (host-side sharding + layout transforms)."""
    xT = np.ascontiguousarray(x.reshape(TOK, HID).T)  # [HID, TOK]

    inv_freq = 1.0 / (10000.0 ** (np.arange(0, HD, 2, dtype=np.float32) / HD))
    freqs = np.arange(S, dtype=np.float32)[:, None] * inv_freq  # [S, 64]
    c = np.cos(freqs).T.astype(np.float32)   # [64, S]
    sn = np.sin(freqs).T.astype(np.float32)
    cosT = np.ascontiguousarray(np.concatenate([c, c], axis=0))       # [cos;cos]
    sinT = np.ascontiguousarray(np.concatenate([-sn, sn], axis=0))    # [-sin;sin]

    import ml_dtypes
    in_maps = []
    for c in range(NCORES):
        wq_c = wq[c * QD:(c + 1) * QD, :]          # [512, HID]
        wk_c = wk[c * HD:(c + 1) * HD, :]          # [128, HID]
        wv_c = wv[c * HD:(c + 1) * HD, :]          # [128, HID]
        wo_c = wo[:, c * QD:(c + 1) * QD]          # [HID, 512]

        # slabs: per (block, ktile) rows [128 hid] x cols [x 512 | wk 128 | wv 128]
        slab = np.empty((B * NBLK * KTILES * 128, 768), np.float32)
        wkT_c = wk_c.T  # [HID, 128]
        wvT_c = wv_c.T
        for blk in range(B * NBLK):
            t0 = blk * BLK
            for k in range(KTILES):
                r0 = (blk * KTILES + k) * 128
                slab[r0:r0 + 128, 0:512] = xT[k * 128:(k + 1) * 128, t0:t0 + BLK]
                slab[r0:r0 + 128, 512:640] = wkT_c[k * 128:(k + 1) * 128, :]
                slab[r0:r0 + 128, 640:768] = wvT_c[k * 128:(k + 1) * 128, :]

        in_maps.append({
            "slabs": slab,
            "wqT": np.ascontiguousarray(wq_c.T),
            "woT": np.ascontiguousarray(wo_c.T).astype(ml_dtypes.bfloat16),
            "cosT": cosT, "sinT": sinT,
            "qnw": q_norm_w.reshape(128, 1).astype(np.float32),
            "knw": k_norm_w.reshape(128, 1).astype(np.float32),
        })
    return in_maps


def kernel(x, wq, wk, wv, wo, q_norm_w, k_norm_w, _trace=False):
    from concourse import bass_utils

    x = np.asarray(x, np.float32)
    wq, wk, wv, wo = (np.asarray(a, np.float32) for a in (wq, wk, wv, wo))
    q_norm_w = np.asarray(q_norm_w, np.float32)
    k_norm_w = np.asarray(k_norm_w, np.float32)

    if "nc" not in _CACHE:
        _CACHE["nc"] = _build()
    nc = _CACHE["nc"]

    in_maps = _host_inputs(x, wq, wk, wv, wo, q_norm_w, k_norm_w)
    res = bass_utils.run_bass_kernel_spmd(
        nc, in_maps, core_ids=list(range(NCORES)), trace=_trace)
    _CACHE["last_result"] = res

    acc = np.zeros((HID, TOK), np.float64)
    for c in range(NCORES):
        acc += res.results[c]["outT"].astype(np.float64)
    out = acc.astype(np.float32).T.reshape(B, S, HID)
    return out
